# revision 7
# baseline (speedup 1.0000x reference)
"""Trainium2 Bass kernel for hierarchical 1D attention (HAttention1D).

Sharding: 8 cores = 4 batches x 2 sequence halves. Each core computes its
half's full pipeline: qkv projection, 9-level block-attention pyramid,
hierarchical combine, output projection. The only cross-half coupling is the
coarsest level (16 positions), supplied via a host-computed 256-mean-pool of
the other half's x (linearity of the projection makes this exact).

Transfer-optimized for the axon tunnel (the dispatch is transfer-bound):
 - x is uploaded int8 with global per-feature scales folded into the qkv
   weights host-side (the 32 pooled cross-half columns ride as a small bf16
   sidecar, pre-divided by the same scales),
 - weights are uploaded as 1/8 shards and AllGathered on-device,
 - the output is stored int8 with a per-row f32 scale (absmax/127, RNE
   rounding on the DVE), dequantized host-side after the gather; this also
   halves the donated zero-output upload run_bass_via_pjrt performs,
 - score masks and the pair-pool matrix ride inside the NEFF as Const
   tensors, and the (zero) output bias is applied host-side.

All matmuls bf16 with fp32 PSUM accumulation. The pair-flip of k/v blocks at
coarse levels is absorbed into constant mask patterns (block-diagonal vs
pair-swapped) added to the raw scores before exp; junk entries of the
all-pairs 128x128 score tile get -1e30 and vanish. Softmax row-max runs in
q-orientation; A^T for the A@V matmul comes from the DVE 32x32 stream
transpose, exact because the valid-block pattern is 32-block symmetric.
"""

import sys

sys.path.insert(0, "/opt/trn_rl_repo")

import functools
import numpy as np
import ml_dtypes

import jax

jax.config.update("jax_compilation_cache_dir", "/tmp/jax_comp_cache")
jax.config.update("jax_persistent_cache_min_compile_time_secs", 0.0)
jax.config.update("jax_persistent_cache_min_entry_size_bytes", 0)

import concourse.bass as bass
import concourse.mybir as mybir
import concourse.tile as tile
from concourse import bacc
from concourse.bass_utils import run_bass_kernel_spmd

BF16 = mybir.dt.bfloat16
F16 = mybir.dt.float16
F32 = mybir.dt.float32
I8 = mybir.dt.int8
AOP = mybir.AluOpType
AFT = mybir.ActivationFunctionType
AXX = mybir.AxisListType

HEADS = 16
DH = 64
BSZ = 16
N = 4096                      # per-core half length
NEXT = 4128                   # + 16 xpool, 16 xpool*256
MS = [4096, 2048, 1024, 512, 256, 128, 64, 32, 16]   # level sizes l=0..8
PYR_OFF = {}                  # col offsets of levels 1..8 in qT/kT tiles
_o = NEXT
for _l in range(1, 9):
    PYR_OFF[_l] = _o
    _o += MS[_l]
LTOT = _o + 16                # + 16 zero pad after l8 (for padded l8 stationary)
SLOT_BASE = {0: 0, 1: 32, 2: 48, 3: 56, 4: 60, 5: 62, 6: 63, 7: 64, 8: 65}
NSLOT = 66
PCHUNKS = [(i * 512, 512) for i in range(8)] + [(4096, 32)]   # proj moving chunks
RG = [[0, 1, 2, 3, 4, 5, 6, 7]]


def _const_arrays():
    bf = ml_dtypes.bfloat16
    mdiag = np.full((128, 128), -1e30, np.float32)
    mflip = np.full((128, 128), -1e30, np.float32)
    for b in range(8):
        mdiag[b * 16:(b + 1) * 16, b * 16:(b + 1) * 16] = 0.0
        p = b ^ 1
        mflip[b * 16:(b + 1) * 16, p * 16:(p + 1) * 16] = 0.0
    pool = np.zeros((128, 64), np.float32)
    for i in range(128):
        pool[i, i // 2] = 1.0
    return mdiag, mflip, pool.astype(bf)


def build_nc():
    nc = bacc.Bacc(None, target_bir_lowering=False, num_devices=8)

    xt8 = nc.dram_tensor("xt8", [1024, N], I8, kind="ExternalInput")
    xext = nc.dram_tensor("xext", [1024, 32], BF16, kind="ExternalInput")
    wqk_sh = nc.dram_tensor("wqk_sh", [128, HEADS, 128], BF16, kind="ExternalInput")
    wv_sh = nc.dram_tensor("wv_sh", [128, 1024], BF16, kind="ExternalInput")
    wout_sh = nc.dram_tensor("wout_sh", [128, 1024], BF16, kind="ExternalInput")
    # cols 0:1024 int8 row-quantized values; cols 1024:1028 the f32 row
    # scale bitcast to 4 bytes (keeps the dispatch at one output tensor —
    # each extra output pays a ~90ms per-array fetch through the tunnel).
    out = nc.dram_tensor("out", [N, 1028], I8, kind="ExternalOutput")

    mdiag_np, mflip_np, pool_np = _const_arrays()
    mdiag = nc.inline_tensor(mdiag_np, name="mdiag")
    mflip = nc.inline_tensor(mflip_np, name="mflip")
    pool = nc.inline_tensor(pool_np, name="pool")

    with tile.TileContext(nc) as tc:
        with (
            tc.tile_pool(name="cst", bufs=1) as cst,
            tc.tile_pool(name="dram", bufs=1, space="DRAM") as dram,
            tc.tile_pool(name="work", bufs=2) as work,
            tc.tile_pool(name="ps_proj", bufs=2, space="PSUM") as ps_proj,
            tc.tile_pool(name="ps_s", bufs=2, space="PSUM") as ps_s,
            tc.tile_pool(name="ps_y", bufs=2, space="PSUM") as ps_y,
            tc.tile_pool(name="ps_vp", bufs=2, space="PSUM") as ps_vp,
        ):
            # ---------- P0: AllGather the weight shards across 8 cores ----
            wqk_bn = dram.tile([128, HEADS, 128], BF16)
            wv_bn = dram.tile([128, 1024], BF16)
            wout_bn = dram.tile([128, 1024], BF16)
            wqk = dram.tile([1024, HEADS, 128], BF16, addr_space="Shared")
            wv = dram.tile([1024, 1024], BF16, addr_space="Shared")
            wout = dram.tile([1024, 1024], BF16, addr_space="Shared")
            nc.gpsimd.dma_start(wqk_bn[:], wqk_sh[:])
            nc.gpsimd.dma_start(wv_bn[:], wv_sh[:])
            nc.gpsimd.dma_start(wout_bn[:], wout_sh[:])
            nc.gpsimd.collective_compute(
                "AllGather", AOP.bypass, replica_groups=RG,
                ins=[wqk_bn.opt()], outs=[wqk.opt()],
            )
            nc.gpsimd.collective_compute(
                "AllGather", AOP.bypass, replica_groups=RG,
                ins=[wv_bn.opt()], outs=[wv.opt()],
            )
            nc.gpsimd.collective_compute(
                "AllGather", AOP.bypass, replica_groups=RG,
                ins=[wout_bn.opt()], outs=[wout.opt()],
            )

            vnat_dram = dram.tile([NEXT, 1024], BF16)
            ynt_dram = dram.tile([1024, N], BF16)

            mdiag_sb = cst.tile([128, 128], F32)
            mflip_sb = cst.tile([128, 128], F32)
            pool_sb = cst.tile([128, 64], BF16)
            nc.sync.dma_start(mdiag_sb[:], mdiag[:])
            nc.sync.dma_start(mflip_sb[:], mflip[:])
            nc.sync.dma_start(pool_sb[:], pool[:])

            with tc.tile_pool(name="p12", bufs=1) as p12:
                import contextlib

                xt_sb = p12.tile([128, 8, NEXT], BF16)
                x8s = contextlib.ExitStack()
                x8pool = x8s.enter_context(tc.tile_pool(name="x8p", bufs=1))
                xi8_sb = x8pool.tile([128, 8, N], I8)
                nc.sync.dma_start(
                    xi8_sb[:], xt8.rearrange("(kt p) n -> p kt n", p=128)
                )
                nc.vector.tensor_copy(xt_sb[:, :, 0:N], xi8_sb[:])
                x8s.close()
                nc.sync.dma_start(
                    xt_sb[:, :, N:NEXT],
                    xext.rearrange("(kt p) e -> p kt e", p=128),
                )

                # ---------- P1: v projection (all heads), pos-major ----------
                wvp = contextlib.ExitStack()
                wvpool = wvp.enter_context(tc.tile_pool(name="wvp", bufs=1))
                wv_sb = wvpool.tile([128, 8, 1024], BF16)
                nc.sync.dma_start(
                    wv_sb[:], wv[:].rearrange("(kt p) n -> p kt n", p=128)
                )
                for pt in range(N // 128):
                    for c in range(2):
                        vps = ps_proj.tile([128, 512], F32, tag="psproj")
                        for kt in range(8):
                            nc.tensor.matmul(
                                vps[:],
                                xt_sb[:, kt, pt * 128:(pt + 1) * 128],
                                wv_sb[:, kt, c * 512:(c + 1) * 512],
                                start=(kt == 0), stop=(kt == 7),
                            )
                        vstage = work.tile([128, 512], BF16, tag="vstage")
                        nc.vector.tensor_copy(vstage[:], vps[:])
                        nc.sync.dma_start(
                            vnat_dram[pt * 128:(pt + 1) * 128,
                                      c * 512:(c + 1) * 512],
                            vstage[:],
                        )
                # v of xpool*256 rows (coarsest-level cross-half v), 16 rows
                for c in range(2):
                    vps = ps_proj.tile([128, 512], F32, tag="psproj")
                    for kt in range(8):
                        nc.tensor.matmul(
                            vps[0:16, :],
                            xt_sb[:, kt, N + 16:N + 32],
                            wv_sb[:, kt, c * 512:(c + 1) * 512],
                            start=(kt == 0), stop=(kt == 7),
                        )
                    vstage = work.tile([128, 512], BF16, tag="vstage")
                    nc.vector.tensor_copy(vstage[0:16, :], vps[0:16, :])
                    nc.sync.dma_start(
                        vnat_dram[N + 16:N + 32, c * 512:(c + 1) * 512],
                        vstage[0:16, :],
                    )

                wvp.close()

                # ---------- P2: per-head projection + attention ----------
                p2stack = contextlib.ExitStack()
                accp = p2stack.enter_context(tc.tile_pool(name="accp", bufs=1))
                divp = p2stack.enter_context(tc.tile_pool(name="divp", bufs=1))
                for h in range(HEADS):
                    wqk_h = work.tile([128, 8, 128], BF16, tag="wqkh")
                    nc.sync.dma_start(
                        wqk_h[:],
                        wqk[:, h, :].rearrange("(kt p) c -> p kt c", p=128),
                    )
                    qT = accp.tile([64, LTOT], BF16, tag="qT")
                    kT = accp.tile([64, LTOT], BF16, tag="kT")
                    # zero the l8 stationary pad region (cols l8..l8+32)
                    nc.vector.memset(qT[:, PYR_OFF[8] + 16:PYR_OFF[8] + 32], 0.0)

                    for (coff, clen) in PCHUNKS:
                        qkps = ps_proj.tile([128, 512], F32, tag="psproj")
                        for kt in range(8):
                            nc.tensor.matmul(
                                qkps[:, :clen],
                                wqk_h[:, kt, :],
                                xt_sb[:, kt, coff:coff + clen],
                                start=(kt == 0), stop=(kt == 7),
                            )
                        nc.vector.tensor_copy(qT[:, coff:coff + clen],
                                              qkps[0:64, :clen])
                        nc.vector.tensor_copy(kT[:, coff:coff + clen],
                                              qkps[64:128, :clen])

                    # q/k sum-pool pyramids (free-dim pairwise adds)
                    for l in range(1, 9):
                        m = MS[l]
                        for t in (qT, kT):
                            src = (t[:, 0:4096] if l == 1
                                   else t[:, PYR_OFF[l - 1]:PYR_OFF[l - 1] + MS[l - 1]])
                            sv = src.rearrange("p (m two) -> p m two", two=2)
                            nc.vector.tensor_tensor(
                                t[:, PYR_OFF[l]:PYR_OFF[l] + m],
                                sv[:, :, 0], sv[:, :, 1], AOP.add,
                            )

                    # vext slots: [128, slot, 128] = [v | ones]
                    vext = accp.tile([128, NSLOT, 128], BF16, tag="vext")
                    nc.vector.memset(vext[:, :, 64:128], 1.0)
                    nc.vector.memset(vext[:, 63:66, 0:64], 0.0)
                    nc.sync.dma_start(
                        vext[:, 0:32, 0:64],
                        vnat_dram[0:4096, h * 64:(h + 1) * 64].rearrange(
                            "(g p) d -> p g d", p=128),
                    )
                    nc.sync.dma_start(
                        vext[0:16, 65, 0:64],
                        vnat_dram[N + 16:N + 32, h * 64:(h + 1) * 64],
                    )
                    # v pyramids via PE pooling
                    for l in range(1, 8):
                        m = MS[l]
                        if m >= 128:
                            for c in range(m // 128):
                                for half in range(2):
                                    pps = ps_vp.tile([64, 64], F32, tag="psvp")
                                    nc.tensor.matmul(
                                        pps[:],
                                        pool_sb[:, 0:64],
                                        vext[:, SLOT_BASE[l - 1] + 2 * c + half, 0:64],
                                        start=True, stop=True,
                                    )
                                    nc.vector.tensor_copy(
                                        vext[half * 64:(half + 1) * 64,
                                             SLOT_BASE[l] + c, 0:64],
                                        pps[:],
                                    )
                        else:
                            msrc = MS[l - 1]
                            pps = ps_vp.tile([64, 64], F32, tag="psvp")
                            nc.tensor.matmul(
                                pps[0:m, :],
                                pool_sb[0:msrc, 0:m],
                                vext[0:msrc, SLOT_BASE[l - 1], 0:64],
                                start=True, stop=True,
                            )
                            nc.vector.tensor_copy(
                                vext[0:m, SLOT_BASE[l], 0:64], pps[0:m, :])

                    # attention slots, coarse -> fine
                    yaccs = {}
                    for l in range(8, -1, -1):
                        m = min(MS[l], 128) if l < 8 else 32
                        nslots = max(MS[l] // 128, 1) if l < 8 else 1
                        mvalid = MS[l] if l < 8 else 16
                        yacc = accp.tile([128, MS[l] if l < 8 else 16],
                                         F32, tag=f"yacc{l}")
                        yaccs[l] = yacc
                        scale = (0.25 ** l) / 8.0 if l < 8 else (0.5 ** 8) / 8.0
                        if l == 0 or l == 8:
                            msk = mdiag_sb
                        else:
                            msk = mflip_sb
                        for g in range(nslots):
                            if l < 8:
                                qoff = (128 * g if l == 0
                                        else PYR_OFF[l] + 128 * g)
                                koff = qoff
                            else:
                                qoff = PYR_OFF[8]
                                koff = 4096
                            sps = ps_s.tile([128, 128], F32, tag="pss")
                            nc.tensor.matmul(
                                sps[0:m, 0:m],
                                qT[:, qoff:qoff + m],
                                kT[:, koff:koff + m],
                                start=True, stop=True,
                            )
                            sm = work.tile([128, 128], F32, tag="sm")
                            nc.vector.tensor_tensor(
                                sm[0:m, 0:m], sps[0:m, 0:m],
                                msk[0:m, 0:m], AOP.add)
                            negb = work.tile([128, 1], F32, tag="negb")
                            nc.vector.reduce_max(
                                negb[0:m, :], sm[0:m, 0:m],
                                axis=AXX.X, negate=True)
                            nsc = work.tile([128, 1], F32, tag="nsc")
                            nc.vector.tensor_scalar_mul(
                                nsc[0:m, :], negb[0:m, :], scale)
                            ab = work.tile([128, 128], BF16, tag="ab")
                            nc.scalar.activation(
                                ab[0:m, 0:m], sm[0:m, 0:m], AFT.Exp,
                                bias=nsc[0:m, :], scale=scale)
                            atb = work.tile([128, 128], BF16, tag="atb")
                            nc.vector.transpose(atb[0:m, 0:m], ab[0:m, 0:m])
                            yps = ps_y.tile([128, 128], F32, tag="psy")
                            slot = SLOT_BASE[l] + g
                            nc.tensor.matmul(
                                yps[:, 0:m],
                                vext[0:m, slot, :],
                                atb[0:m, 0:m],
                                start=True, stop=True,
                            )
                            # telescope
                            mv = min(mvalid, 128)
                            if l == 8:
                                nc.vector.tensor_copy(yacc[:, 0:16],
                                                      yps[:, 0:16])
                            else:
                                half = mv // 2
                                prev = yaccs[l + 1]
                                pv = prev[:, half * g:half * g + half]
                                nc.vector.tensor_tensor(
                                    yacc[:, mv * g:mv * g + mv].rearrange(
                                        "p (a b) -> p a b", b=2),
                                    yps[:, 0:mv].rearrange(
                                        "p (a b) -> p a b", b=2),
                                    pv[:, :, None].to_broadcast(
                                        (128, half, 2)),
                                    AOP.add,
                                )

                    # divide + store ynT (chunked)
                    y0 = yaccs[0]
                    ynt = divp.tile([64, 4096], BF16, tag="ynt")
                    for dc in range(4):
                        sl = slice(dc * 1024, (dc + 1) * 1024)
                        arow = divp.tile([64, 1024], F32, tag="arow")
                        nc.vector.tensor_copy(arow[:], y0[64:128, sl])
                        rcp = divp.tile([64, 1024], F32, tag="rcp")
                        nc.vector.reciprocal(rcp[:], arow[:])
                        nc.vector.tensor_tensor(ynt[:, sl], y0[0:64, sl],
                                                rcp[:], AOP.mult)
                    nc.sync.dma_start(
                        ynt_dram[h * 64:(h + 1) * 64, :], ynt[:])

                p2stack.close()
            # ---------- P3: output projection + int8 row quantization ------
            with tc.tile_pool(name="p3", bufs=1) as p3:
                ynt_sb = p3.tile([128, 8, N], BF16)
                nc.sync.dma_start(
                    ynt_sb[:], ynt_dram[:].rearrange("(kt p) n -> p kt n", p=128))
                wout_sb = p3.tile([128, 8, 1024], BF16)
                nc.sync.dma_start(
                    wout_sb[:], wout[:].rearrange("(kt p) n -> p kt n", p=128))
                for pt in range(32):
                    orow = work.tile([128, 1024], F32, tag="orow")
                    for c in range(2):
                        ops = ps_proj.tile([128, 512], F32, tag="psproj")
                        for kt in range(8):
                            nc.tensor.matmul(
                                ops[:],
                                ynt_sb[:, kt, pt * 128:(pt + 1) * 128],
                                wout_sb[:, kt, c * 512:(c + 1) * 512],
                                start=(kt == 0), stop=(kt == 7),
                            )
                        nc.vector.tensor_copy(
                            orow[:, c * 512:(c + 1) * 512], ops[:])
                    amax = work.tile([128, 1], F32, tag="amax")
                    nc.vector.tensor_reduce(
                        amax[:], orow[:], op=AOP.max, axis=AXX.X,
                        apply_absolute_value=True)
                    nc.vector.tensor_scalar_max(amax[:], amax[:], 1e-30)
                    rcq = work.tile([128, 1], F32, tag="rcq")
                    nc.vector.reciprocal(rcq[:], amax[:])
                    qf = work.tile([128, 1024], F32, tag="qf")
                    nc.vector.tensor_scalar(
                        qf[:], orow[:], rcq[:], 127.0, AOP.mult, AOP.mult)
                    qi = work.tile([128, 1024], I8, tag="qi")
                    nc.vector.tensor_copy(qi[:], qf[:])
                    nc.sync.dma_start(
                        out[pt * 128:(pt + 1) * 128, 0:1024], qi[:])
                    sct = work.tile([128, 1], F32, tag="sct")
                    nc.vector.tensor_scalar_mul(sct[:], amax[:], 1.0 / 127.0)
                    nc.sync.dma_start(
                        out[pt * 128:(pt + 1) * 128, 1024:1028],
                        sct[:].bitcast(I8))

    nc.compile()
    # The BIR is immutable after compile(), but run_bass_via_pjrt re-lowers
    # (and re-serializes) it on every dispatch; memoize the 20MB JSON dump.
    bir_bytes = nc.to_json_bytes()
    nc.to_json_bytes = lambda: bir_bytes
    return nc


@functools.lru_cache(maxsize=1)
def _cached_nc():
    return build_nc()


def _host_inputs(x, w_qkv, w_out, b_out):
    bf = ml_dtypes.bfloat16
    # global per-feature int8 scales for x, folded into the qkv weights
    sx = np.abs(x).reshape(-1, 1024).max(axis=0) / 127.0   # [1024]
    sx = np.maximum(sx, 1e-30)
    wqkv_f = w_qkv * sx[:, None]
    wq, wk, wv = wqkv_f[:, 0:1024], wqkv_f[:, 1024:2048], wqkv_f[:, 2048:3072]
    wqk = np.empty((1024, HEADS, 128), dtype=np.float32)
    for h in range(HEADS):
        wqk[:, h, 0:64] = wq[:, h * 64:(h + 1) * 64]
        wqk[:, h, 64:128] = wk[:, h * 64:(h + 1) * 64]
    wqk = np.ascontiguousarray(wqk.astype(bf))
    wvb = np.ascontiguousarray(wv.astype(bf))
    woutb = np.ascontiguousarray(w_out.astype(bf))
    rsx = (1.0 / sx)[:, None]

    in_maps = []
    for core in range(8):
        b, s = core // 2, core % 2
        xh = x[b, s * N:(s + 1) * N, :]
        xo = x[b, (1 - s) * N:(2 - s) * N, :]
        xpool = xo.reshape(16, 256, 1024).mean(axis=1)
        xi8 = np.rint(xh.T * rsx).clip(-127, 127).astype(np.int8)
        xe = np.empty((1024, 32), np.float32)
        xe[:, 0:16] = xpool.T * rsx
        xe[:, 16:32] = xpool.T * (256.0 * rsx)
        r = slice(core * 128, (core + 1) * 128)
        in_maps.append(dict(
            xt8=np.ascontiguousarray(xi8), xext=xe.astype(bf),
            wqk_sh=wqk[r], wv_sh=wvb[r], wout_sh=woutb[r],
        ))
    return in_maps


def kernel(x, w_qkv, w_out, b_out):
    nc = _cached_nc()
    in_maps = _host_inputs(np.asarray(x, np.float32), np.asarray(w_qkv, np.float32),
                           np.asarray(w_out, np.float32), np.asarray(b_out, np.float32))
    res = run_bass_kernel_spmd(nc, in_maps, core_ids=list(range(8)))
    bias = np.asarray(b_out, np.float32)
    out = np.empty((4, 8192, 1024), np.float32)
    for core in range(8):
        b, s = core // 2, core % 2
        raw = res.results[core]["out"]
        scales = np.ascontiguousarray(raw[:, 1024:1028]).view(np.float32)
        blk = raw[:, 0:1024].astype(np.float32)
        blk *= scales
        blk += bias
        out[b, s * N:(s + 1) * N, :] = blk
    return out


# revision 12
# speedup vs baseline: 1.0174x; 1.0174x over previous
"""Trainium2 Bass kernel for hierarchical 1D attention (HAttention1D).

Sharding: 8 cores = 4 batches x 2 sequence halves. Each core computes its
half's full pipeline: qkv projection, 9-level block-attention pyramid,
hierarchical combine, output projection. The only cross-half coupling is the
coarsest level (16 positions), supplied via a host-computed 256-mean-pool of
the other half's x (linearity of the projection makes this exact).

Transfer-optimized for the axon tunnel (the dispatch is transfer-bound):
 - x is uploaded int8 with global per-feature scales folded into the qkv
   weights host-side (the 32 pooled cross-half columns ride as a small bf16
   sidecar, pre-divided by the same scales),
 - weights are uploaded as 1/8 shards and AllGathered on-device,
 - the output is stored int8 with a per-row f32 scale (absmax/127, RNE
   rounding on the DVE), dequantized host-side after the gather; this also
   halves the donated zero-output upload run_bass_via_pjrt performs,
 - score masks and the pair-pool matrix ride inside the NEFF as Const
   tensors, and the (zero) output bias is applied host-side.

All matmuls bf16 with fp32 PSUM accumulation. The pair-flip of k/v blocks at
coarse levels is absorbed into constant mask patterns (block-diagonal vs
pair-swapped) added to the raw scores before exp; junk entries of the
all-pairs 128x128 score tile get -1e30 and vanish. Softmax row-max runs in
q-orientation; A^T for the A@V matmul comes from the DVE 32x32 stream
transpose, exact because the valid-block pattern is 32-block symmetric.
"""

import sys

sys.path.insert(0, "/opt/trn_rl_repo")

import functools
import numpy as np
import ml_dtypes

import jax

jax.config.update("jax_compilation_cache_dir", "/tmp/jax_comp_cache")
jax.config.update("jax_persistent_cache_min_compile_time_secs", 0.0)
jax.config.update("jax_persistent_cache_min_entry_size_bytes", 0)

import concourse.bass as bass
import concourse.mybir as mybir
import concourse.tile as tile
from concourse import bacc
from concourse.bass_utils import run_bass_kernel_spmd

BF16 = mybir.dt.bfloat16
F16 = mybir.dt.float16
F32 = mybir.dt.float32
I8 = mybir.dt.int8
AOP = mybir.AluOpType
AFT = mybir.ActivationFunctionType
AXX = mybir.AxisListType

HEADS = 16
DH = 64
BSZ = 16
N = 4096                      # per-core half length
NEXT = 4128                   # + 16 xpool, 16 xpool*256
MS = [4096, 2048, 1024, 512, 256, 128, 64, 32, 16]   # level sizes l=0..8
PYR_OFF = {}                  # col offsets of levels 1..8 in qT/kT tiles
_o = NEXT
for _l in range(1, 9):
    PYR_OFF[_l] = _o
    _o += MS[_l]
LTOT = _o + 16                # + 16 zero pad after l8 (for padded l8 stationary)
SLOT_BASE = {0: 0, 1: 32, 2: 48, 3: 56, 4: 60, 5: 62, 6: 63, 7: 64, 8: 65}
NSLOT = 66
PCHUNKS = [(i * 512, 512) for i in range(8)] + [(4096, 32)]   # proj moving chunks
RG = [[0, 1, 2, 3, 4, 5, 6, 7]]
# byte offsets of the sections inside the packed per-core input
PK_XT8 = 0                                  # int8 [1024, 4096]
PK_XEXT = PK_XT8 + 1024 * N                 # bf16 [1024, 32]
PK_WQK = PK_XEXT + 1024 * 32 * 2            # bf16 [128, 16, 128]
PK_WV = PK_WQK + 128 * HEADS * 128 * 2      # bf16 [128, 1024]
PK_WOUT = PK_WV + 128 * 1024 * 2            # bf16 [128, 1024]
PACK_BYTES = PK_WOUT + 128 * 1024 * 2


def _const_arrays():
    bf = ml_dtypes.bfloat16
    mdiag = np.full((128, 128), -1e30, np.float32)
    mflip = np.full((128, 128), -1e30, np.float32)
    for b in range(8):
        mdiag[b * 16:(b + 1) * 16, b * 16:(b + 1) * 16] = 0.0
        p = b ^ 1
        mflip[b * 16:(b + 1) * 16, p * 16:(p + 1) * 16] = 0.0
    pool = np.zeros((128, 64), np.float32)
    for i in range(128):
        pool[i, i // 2] = 1.0
    return mdiag, mflip, pool.astype(bf)


def build_nc():
    nc = bacc.Bacc(None, target_bir_lowering=False, num_devices=8)

    # One packed input tensor per core (each extra array in the dispatch
    # pays ~90ms of per-array tunnel overhead): bytes are
    # [xt8 int8 [1024,4096] | xext bf16 [1024,32] | wqk_sh bf16 [128,16,128]
    #  | wv_sh bf16 [128,1024] | wout_sh bf16 [128,1024]].
    pk = nc.dram_tensor("pk", [PACK_BYTES], I8, kind="ExternalInput")
    # cols 0:1024 int8 row-quantized values; cols 1024:1028 the f32 row
    # scale bitcast to 4 bytes (keeps the dispatch at one output tensor —
    # each extra output pays a ~90ms per-array fetch through the tunnel).
    out = nc.dram_tensor("out", [N, 1028], I8, kind="ExternalOutput")

    mdiag_np, mflip_np, pool_np = _const_arrays()
    mdiag = nc.inline_tensor(mdiag_np, name="mdiag")
    mflip = nc.inline_tensor(mflip_np, name="mflip")
    pool = nc.inline_tensor(pool_np, name="pool")

    with tile.TileContext(nc) as tc:
        with (
            tc.tile_pool(name="cst", bufs=1) as cst,
            tc.tile_pool(name="dram", bufs=1, space="DRAM") as dram,
            tc.tile_pool(name="work", bufs=2) as work,
            tc.tile_pool(name="ps_proj", bufs=2, space="PSUM") as ps_proj,
            tc.tile_pool(name="ps_s", bufs=2, space="PSUM") as ps_s,
            tc.tile_pool(name="ps_y", bufs=2, space="PSUM") as ps_y,
            tc.tile_pool(name="ps_vp", bufs=2, space="PSUM") as ps_vp,
        ):
            # ---------- P0: AllGather the weight shards across 8 cores ----
            wqk_bn = dram.tile([128 * HEADS * 128], BF16)
            wv_bn = dram.tile([128 * 1024], BF16)
            wout_bn = dram.tile([128 * 1024], BF16)
            wqk = dram.tile([1024, HEADS, 128], BF16, addr_space="Shared")
            wv = dram.tile([1024, 1024], BF16, addr_space="Shared")
            wout = dram.tile([1024, 1024], BF16, addr_space="Shared")
            nc.gpsimd.dma_start(wqk_bn[:], pk[PK_WQK:PK_WV].bitcast(BF16))
            nc.gpsimd.dma_start(wv_bn[:], pk[PK_WV:PK_WOUT].bitcast(BF16))
            nc.gpsimd.dma_start(wout_bn[:], pk[PK_WOUT:PACK_BYTES].bitcast(BF16))
            nc.gpsimd.collective_compute(
                "AllGather", AOP.bypass, replica_groups=RG,
                ins=[wqk_bn.opt()], outs=[wqk.opt()],
            )
            nc.gpsimd.collective_compute(
                "AllGather", AOP.bypass, replica_groups=RG,
                ins=[wv_bn.opt()], outs=[wv.opt()],
            )
            nc.gpsimd.collective_compute(
                "AllGather", AOP.bypass, replica_groups=RG,
                ins=[wout_bn.opt()], outs=[wout.opt()],
            )

            vnat_dram = dram.tile([NEXT, 1024], BF16)
            ynt_dram = dram.tile([1024, N], BF16)

            mdiag_sb = cst.tile([128, 128], F32)
            mflip_sb = cst.tile([128, 128], F32)
            pool_sb = cst.tile([128, 64], BF16)
            nc.sync.dma_start(mdiag_sb[:], mdiag[:])
            nc.sync.dma_start(mflip_sb[:], mflip[:])
            nc.sync.dma_start(pool_sb[:], pool[:])

            with tc.tile_pool(name="p12", bufs=1) as p12:
                import contextlib

                xt_sb = p12.tile([128, 8, NEXT], BF16)
                x8s = contextlib.ExitStack()
                x8pool = x8s.enter_context(tc.tile_pool(name="x8p", bufs=1))
                xi8_sb = x8pool.tile([128, 8, N], I8)
                nc.sync.dma_start(
                    xi8_sb[:],
                    pk[PK_XT8:PK_XEXT].rearrange(
                        "(kt p n) -> p kt n", p=128, n=N),
                )
                nc.vector.tensor_copy(xt_sb[:, :, 0:N], xi8_sb[:])
                x8s.close()
                nc.sync.dma_start(
                    xt_sb[:, :, N:NEXT],
                    pk[PK_XEXT:PK_WQK].bitcast(BF16).rearrange(
                        "(kt p e) -> p kt e", p=128, e=32),
                )

                # ---------- P1: v projection (all heads), pos-major ----------
                wvp = contextlib.ExitStack()
                wvpool = wvp.enter_context(tc.tile_pool(name="wvp", bufs=1))
                wv_sb = wvpool.tile([128, 8, 1024], BF16)
                nc.sync.dma_start(
                    wv_sb[:], wv[:].rearrange("(kt p) n -> p kt n", p=128)
                )
                for pt in range(N // 128):
                    for c in range(2):
                        vps = ps_proj.tile([128, 512], F32, tag="psproj")
                        for kt in range(8):
                            nc.tensor.matmul(
                                vps[:],
                                xt_sb[:, kt, pt * 128:(pt + 1) * 128],
                                wv_sb[:, kt, c * 512:(c + 1) * 512],
                                start=(kt == 0), stop=(kt == 7),
                            )
                        vstage = work.tile([128, 512], BF16, tag="vstage")
                        nc.vector.tensor_copy(vstage[:], vps[:])
                        nc.sync.dma_start(
                            vnat_dram[pt * 128:(pt + 1) * 128,
                                      c * 512:(c + 1) * 512],
                            vstage[:],
                        )
                # v of xpool*256 rows (coarsest-level cross-half v), 16 rows
                for c in range(2):
                    vps = ps_proj.tile([128, 512], F32, tag="psproj")
                    for kt in range(8):
                        nc.tensor.matmul(
                            vps[0:16, :],
                            xt_sb[:, kt, N + 16:N + 32],
                            wv_sb[:, kt, c * 512:(c + 1) * 512],
                            start=(kt == 0), stop=(kt == 7),
                        )
                    vstage = work.tile([128, 512], BF16, tag="vstage")
                    nc.vector.tensor_copy(vstage[0:16, :], vps[0:16, :])
                    nc.sync.dma_start(
                        vnat_dram[N + 16:N + 32, c * 512:(c + 1) * 512],
                        vstage[0:16, :],
                    )

                wvp.close()

                # ---------- P2: per-head projection + attention ----------
                p2stack = contextlib.ExitStack()
                accp = p2stack.enter_context(tc.tile_pool(name="accp", bufs=1))
                divp = p2stack.enter_context(tc.tile_pool(name="divp", bufs=1))
                for h in range(HEADS):
                    wqk_h = work.tile([128, 8, 128], BF16, tag="wqkh")
                    nc.sync.dma_start(
                        wqk_h[:],
                        wqk[:, h, :].rearrange("(kt p) c -> p kt c", p=128),
                    )
                    qT = accp.tile([64, LTOT], BF16, tag="qT")
                    kT = accp.tile([64, LTOT], BF16, tag="kT")
                    # zero the l8 stationary pad region (cols l8..l8+32)
                    nc.vector.memset(qT[:, PYR_OFF[8] + 16:PYR_OFF[8] + 32], 0.0)

                    for (coff, clen) in PCHUNKS:
                        qkps = ps_proj.tile([128, 512], F32, tag="psproj")
                        for kt in range(8):
                            nc.tensor.matmul(
                                qkps[:, :clen],
                                wqk_h[:, kt, :],
                                xt_sb[:, kt, coff:coff + clen],
                                start=(kt == 0), stop=(kt == 7),
                            )
                        nc.vector.tensor_copy(qT[:, coff:coff + clen],
                                              qkps[0:64, :clen])
                        nc.vector.tensor_copy(kT[:, coff:coff + clen],
                                              qkps[64:128, :clen])

                    # q/k sum-pool pyramids (free-dim pairwise adds)
                    for l in range(1, 9):
                        m = MS[l]
                        for t in (qT, kT):
                            src = (t[:, 0:4096] if l == 1
                                   else t[:, PYR_OFF[l - 1]:PYR_OFF[l - 1] + MS[l - 1]])
                            sv = src.rearrange("p (m two) -> p m two", two=2)
                            nc.vector.tensor_tensor(
                                t[:, PYR_OFF[l]:PYR_OFF[l] + m],
                                sv[:, :, 0], sv[:, :, 1], AOP.add,
                            )

                    # vext slots: [128, slot, 128] = [v | ones]
                    vext = accp.tile([128, NSLOT, 128], BF16, tag="vext")
                    nc.vector.memset(vext[:, :, 64:128], 1.0)
                    nc.vector.memset(vext[:, 63:66, 0:64], 0.0)
                    nc.sync.dma_start(
                        vext[:, 0:32, 0:64],
                        vnat_dram[0:4096, h * 64:(h + 1) * 64].rearrange(
                            "(g p) d -> p g d", p=128),
                    )
                    nc.sync.dma_start(
                        vext[0:16, 65, 0:64],
                        vnat_dram[N + 16:N + 32, h * 64:(h + 1) * 64],
                    )
                    # v pyramids via PE pooling
                    for l in range(1, 8):
                        m = MS[l]
                        if m >= 128:
                            for c in range(m // 128):
                                for half in range(2):
                                    pps = ps_vp.tile([64, 64], F32, tag="psvp")
                                    nc.tensor.matmul(
                                        pps[:],
                                        pool_sb[:, 0:64],
                                        vext[:, SLOT_BASE[l - 1] + 2 * c + half, 0:64],
                                        start=True, stop=True,
                                    )
                                    nc.vector.tensor_copy(
                                        vext[half * 64:(half + 1) * 64,
                                             SLOT_BASE[l] + c, 0:64],
                                        pps[:],
                                    )
                        else:
                            msrc = MS[l - 1]
                            pps = ps_vp.tile([64, 64], F32, tag="psvp")
                            nc.tensor.matmul(
                                pps[0:m, :],
                                pool_sb[0:msrc, 0:m],
                                vext[0:msrc, SLOT_BASE[l - 1], 0:64],
                                start=True, stop=True,
                            )
                            nc.vector.tensor_copy(
                                vext[0:m, SLOT_BASE[l], 0:64], pps[0:m, :])

                    # attention slots, coarse -> fine
                    yaccs = {}
                    for l in range(8, -1, -1):
                        m = min(MS[l], 128) if l < 8 else 32
                        nslots = max(MS[l] // 128, 1) if l < 8 else 1
                        mvalid = MS[l] if l < 8 else 16
                        yacc = accp.tile([128, MS[l] if l < 8 else 16],
                                         F32, tag=f"yacc{l}")
                        yaccs[l] = yacc
                        scale = (0.25 ** l) / 8.0 if l < 8 else (0.5 ** 8) / 8.0
                        if l == 0 or l == 8:
                            msk = mdiag_sb
                        else:
                            msk = mflip_sb
                        for g in range(nslots):
                            if l < 8:
                                qoff = (128 * g if l == 0
                                        else PYR_OFF[l] + 128 * g)
                                koff = qoff
                            else:
                                qoff = PYR_OFF[8]
                                koff = 4096
                            sps = ps_s.tile([128, 128], F32, tag="pss")
                            nc.tensor.matmul(
                                sps[0:m, 0:m],
                                qT[:, qoff:qoff + m],
                                kT[:, koff:koff + m],
                                start=True, stop=True,
                            )
                            sm = work.tile([128, 128], F32, tag="sm")
                            nc.vector.tensor_tensor(
                                sm[0:m, 0:m], sps[0:m, 0:m],
                                msk[0:m, 0:m], AOP.add)
                            negb = work.tile([128, 1], F32, tag="negb")
                            nc.vector.reduce_max(
                                negb[0:m, :], sm[0:m, 0:m],
                                axis=AXX.X, negate=True)
                            nsc = work.tile([128, 1], F32, tag="nsc")
                            nc.vector.tensor_scalar_mul(
                                nsc[0:m, :], negb[0:m, :], scale)
                            ab = work.tile([128, 128], BF16, tag="ab")
                            nc.scalar.activation(
                                ab[0:m, 0:m], sm[0:m, 0:m], AFT.Exp,
                                bias=nsc[0:m, :], scale=scale)
                            atb = work.tile([128, 128], BF16, tag="atb")
                            nc.vector.transpose(atb[0:m, 0:m], ab[0:m, 0:m])
                            yps = ps_y.tile([128, 128], F32, tag="psy")
                            slot = SLOT_BASE[l] + g
                            nc.tensor.matmul(
                                yps[:, 0:m],
                                vext[0:m, slot, :],
                                atb[0:m, 0:m],
                                start=True, stop=True,
                            )
                            # telescope
                            mv = min(mvalid, 128)
                            if l == 8:
                                nc.vector.tensor_copy(yacc[:, 0:16],
                                                      yps[:, 0:16])
                            else:
                                half = mv // 2
                                prev = yaccs[l + 1]
                                pv = prev[:, half * g:half * g + half]
                                nc.vector.tensor_tensor(
                                    yacc[:, mv * g:mv * g + mv].rearrange(
                                        "p (a b) -> p a b", b=2),
                                    yps[:, 0:mv].rearrange(
                                        "p (a b) -> p a b", b=2),
                                    pv[:, :, None].to_broadcast(
                                        (128, half, 2)),
                                    AOP.add,
                                )

                    # divide + store ynT (chunked)
                    y0 = yaccs[0]
                    ynt = divp.tile([64, 4096], BF16, tag="ynt")
                    for dc in range(4):
                        sl = slice(dc * 1024, (dc + 1) * 1024)
                        arow = divp.tile([64, 1024], F32, tag="arow")
                        nc.vector.tensor_copy(arow[:], y0[64:128, sl])
                        rcp = divp.tile([64, 1024], F32, tag="rcp")
                        nc.vector.reciprocal(rcp[:], arow[:])
                        nc.vector.tensor_tensor(ynt[:, sl], y0[0:64, sl],
                                                rcp[:], AOP.mult)
                    nc.sync.dma_start(
                        ynt_dram[h * 64:(h + 1) * 64, :], ynt[:])

                p2stack.close()
            # ---------- P3: output projection + int8 row quantization ------
            with tc.tile_pool(name="p3", bufs=1) as p3:
                ynt_sb = p3.tile([128, 8, N], BF16)
                nc.sync.dma_start(
                    ynt_sb[:], ynt_dram[:].rearrange("(kt p) n -> p kt n", p=128))
                wout_sb = p3.tile([128, 8, 1024], BF16)
                nc.sync.dma_start(
                    wout_sb[:], wout[:].rearrange("(kt p) n -> p kt n", p=128))
                for pt in range(32):
                    orow = work.tile([128, 1024], F32, tag="orow")
                    for c in range(2):
                        ops = ps_proj.tile([128, 512], F32, tag="psproj")
                        for kt in range(8):
                            nc.tensor.matmul(
                                ops[:],
                                ynt_sb[:, kt, pt * 128:(pt + 1) * 128],
                                wout_sb[:, kt, c * 512:(c + 1) * 512],
                                start=(kt == 0), stop=(kt == 7),
                            )
                        nc.vector.tensor_copy(
                            orow[:, c * 512:(c + 1) * 512], ops[:])
                    amax = work.tile([128, 1], F32, tag="amax")
                    nc.vector.tensor_reduce(
                        amax[:], orow[:], op=AOP.max, axis=AXX.X,
                        apply_absolute_value=True)
                    nc.vector.tensor_scalar_max(amax[:], amax[:], 1e-30)
                    rcq = work.tile([128, 1], F32, tag="rcq")
                    nc.vector.reciprocal(rcq[:], amax[:])
                    qf = work.tile([128, 1024], F32, tag="qf")
                    nc.vector.tensor_scalar(
                        qf[:], orow[:], rcq[:], 127.0, AOP.mult, AOP.mult)
                    qi = work.tile([128, 1024], I8, tag="qi")
                    nc.vector.tensor_copy(qi[:], qf[:])
                    nc.sync.dma_start(
                        out[pt * 128:(pt + 1) * 128, 0:1024], qi[:])
                    sct = work.tile([128, 1], F32, tag="sct")
                    nc.vector.tensor_scalar_mul(sct[:], amax[:], 1.0 / 127.0)
                    nc.sync.dma_start(
                        out[pt * 128:(pt + 1) * 128, 1024:1028],
                        sct[:].bitcast(I8))

    nc.compile()
    # The BIR is immutable after compile(), but run_bass_via_pjrt re-lowers
    # (and re-serializes) it on every dispatch; memoize the 20MB JSON dump.
    bir_bytes = nc.to_json_bytes()
    nc.to_json_bytes = lambda: bir_bytes
    return nc


@functools.lru_cache(maxsize=1)
def _cached_nc():
    return build_nc()


def _host_inputs(x, w_qkv, w_out, b_out):
    bf = ml_dtypes.bfloat16
    # global per-feature int8 scales for x, folded into the qkv weights
    sx = np.abs(x).reshape(-1, 1024).max(axis=0) / 127.0   # [1024]
    sx = np.maximum(sx, 1e-30)
    wqkv_f = w_qkv * sx[:, None]
    wq, wk, wv = wqkv_f[:, 0:1024], wqkv_f[:, 1024:2048], wqkv_f[:, 2048:3072]
    wqk = np.empty((1024, HEADS, 128), dtype=np.float32)
    for h in range(HEADS):
        wqk[:, h, 0:64] = wq[:, h * 64:(h + 1) * 64]
        wqk[:, h, 64:128] = wk[:, h * 64:(h + 1) * 64]
    wqk = np.ascontiguousarray(wqk.astype(bf))
    wvb = np.ascontiguousarray(wv.astype(bf))
    woutb = np.ascontiguousarray(w_out.astype(bf))
    rsx = (1.0 / sx)[:, None]

    in_maps = []
    for core in range(8):
        b, s = core // 2, core % 2
        xh = x[b, s * N:(s + 1) * N, :]
        xo = x[b, (1 - s) * N:(2 - s) * N, :]
        xpool = xo.reshape(16, 256, 1024).mean(axis=1)
        xi8 = np.rint(xh.T * rsx).clip(-127, 127).astype(np.int8)
        xe = np.empty((1024, 32), np.float32)
        xe[:, 0:16] = xpool.T * rsx
        xe[:, 16:32] = xpool.T * (256.0 * rsx)
        r = slice(core * 128, (core + 1) * 128)
        pk = np.empty(PACK_BYTES, np.int8)
        pk[PK_XT8:PK_XEXT] = xi8.reshape(-1)
        pk[PK_XEXT:PK_WQK] = xe.astype(bf).view(np.int8).reshape(-1)
        pk[PK_WQK:PK_WV] = wqk[r].view(np.int8).reshape(-1)
        pk[PK_WV:PK_WOUT] = wvb[r].view(np.int8).reshape(-1)
        pk[PK_WOUT:PACK_BYTES] = woutb[r].view(np.int8).reshape(-1)
        in_maps.append(dict(pk=pk))
    return in_maps


def kernel(x, w_qkv, w_out, b_out):
    nc = _cached_nc()
    in_maps = _host_inputs(np.asarray(x, np.float32), np.asarray(w_qkv, np.float32),
                           np.asarray(w_out, np.float32), np.asarray(b_out, np.float32))
    res = run_bass_kernel_spmd(nc, in_maps, core_ids=list(range(8)))
    bias = np.asarray(b_out, np.float32)
    out = np.empty((4, 8192, 1024), np.float32)
    for core in range(8):
        b, s = core // 2, core % 2
        raw = res.results[core]["out"]
        scales = np.ascontiguousarray(raw[:, 1024:1028]).view(np.float32)
        blk = raw[:, 0:1024].astype(np.float32)
        blk *= scales
        blk += bias
        out[b, s * N:(s + 1) * N, :] = blk
    return out
